# revision 28
# baseline (speedup 1.0000x reference)
"""MinkowskiFlow coarse-flow kernel for 8 Trainium2 NeuronCores (Bass/Tile).

Math (per batch b):
    fs = normalize(feat_s); ft = normalize(feat_t)
    C[n,m]   = 2 - 2 <fs_n, ft_m>
    K[n,m]   = exp(-C/(exp(eps)+0.03)) * (||coor_s_n - coor_t_m||^2 < 100)
    out[n,:] = (K @ coor_t) / (sum_m K + 1e-8) - coor_s

Sharding: batch b -> 4 cores each (data parallel over B=2), N split into 4
row blocks of 1024 (row-wise; each row's normalization is independent).

Per-core layout (all "transposed": target index m on SBUF partitions):
  S^T[m,n]   ONE bf16 PE pass per (m-tile, n-chunk): lhsT rows 0:64 hold
             ft_hi^T, rows 64:128 ft_lo^T (exact split of RAW ft), rhs is
             bf16(normalized fs) duplicated on both halves -> sp = ft.fs_hi.
             ft normalization folds into the ACT exp as a per-partition
             scale (2/tau)/|ft_m|; dropping fs_lo perturbs S by ~1.4e-4 rms
             (weight rel err ~0.8%), well inside the 2e-2 gate (measured
             1.3e-4 overall).
  dist mask  R'[m,n] = -2*ctc_m . csc_n + |csc_n|^2 computed as ONE K=21
             bf16 matmul over a 3-way bf16 split (h+m+l) of CENTERED
             coords: terms h.h, cs2(h,m,l), h.m, m.h, m.m, h.l, l.h in
             that accumulation order (large terms first). Knife-edge
             radius pairs carry up to 31% of a row's softmax weight with
             a 6.6e-4 margin; this split keeps |err| ~ 1e-4.
             mask = R' < 100 - |ct-20|^2   (fp32 threshold per partition)
  K^T        = exp(sp*scl_m - 2/tau) * mask, one ACT op + one fused DVE
             scalar_tensor_tensor (is_lt, mult), stored bf16.
  agg        ONE bf16 matmul per tile: lhsT [128,36] holds [ct_hi | 1] in
             cols 0:4 and [ct_lo | 0] in cols 32:36 -> PSUM [36, n];
             rows 0:4 + rows 32:36 added at the end (exact coord split).
Final per n-tile: PE-transpose agg slice, out = acc*recip(rs+1e-8) - coor_s.

All SBUF<->SBUF transposes are single batched xbar instructions (the 3D
out AP makes the DMA transpose each 128-col block in place). Rep-invariant
zero pads are memset once, on the Pool engine; the small coord-row builder
copies also run on Pool to keep DVE (the bottleneck) free.
"""
import numpy as np
from contextlib import ExitStack

import concourse.bass as bass
import concourse.bacc as bacc
import concourse.tile as tile
import concourse.mybir as mybir
from concourse import masks
from concourse.bass_utils import run_bass_kernel_spmd

F32 = mybir.dt.float32
BF16 = mybir.dt.bfloat16
AF = mybir.ActivationFunctionType
ALU = mybir.AluOpType

B, N, M, D = 2, 4096, 4096, 64
N_CORES = 8
CORES_PER_BATCH = N_CORES // B      # 4
NS = N // CORES_PER_BATCH           # 1024 source rows per core
P = 128
MT = M // P                         # 32 target tiles
NT = NS // P                        # 8 source tiles per core
CHUNK = 512
NCHUNK = NS // CHUNK                # 2
CENTER = 20.0
TAU_OFFSET = 0.03
RADIUS_SQ = 100.0
KC = 21                             # coord-matmul contraction rows


def build_kernel(tau: float, repeat: int = 1):
    nc = bacc.Bacc("TRN2", target_bir_lowering=False, debug=False,
                   num_devices=N_CORES)
    fs_d = nc.dram_tensor("fs", [NS, D], F32, kind="ExternalInput").ap()
    ft_d = nc.dram_tensor("ft", [M, D], F32, kind="ExternalInput").ap()
    cs_d = nc.dram_tensor("cs", [NS, 3], F32, kind="ExternalInput").ap()
    ct_d = nc.dram_tensor("ct", [M, 3], F32, kind="ExternalInput").ap()
    out_d = nc.dram_tensor("out", [NS, 3], F32, kind="ExternalOutput").ap()

    scale = float(2.0 / tau)

    with tile.TileContext(nc) as tc, ExitStack() as ctx:
        pers = ctx.enter_context(tc.tile_pool(name="pers", bufs=1))
        scr = ctx.enter_context(tc.tile_pool(name="scr", bufs=3))
        sbE = ctx.enter_context(tc.tile_pool(name="sbE", bufs=3))
        sbK = ctx.enter_context(tc.tile_pool(name="sbK", bufs=3))
        fin = ctx.enter_context(tc.tile_pool(name="fin", bufs=2))
        psA = ctx.enter_context(tc.tile_pool(name="psA", bufs=3, space="PSUM"))
        psB = ctx.enter_context(tc.tile_pool(name="psB", bufs=3, space="PSUM"))
        psG = ctx.enter_context(tc.tile_pool(name="psG", bufs=1, space="PSUM"))
        psS = ctx.enter_context(tc.tile_pool(name="psS", bufs=1, space="PSUM"))

        # ---------------- persistent tensors ----------------
        ftT = pers.tile([P, M], BF16)       # rows 0:64 ft_hi^T, 64:128 ft_lo^T
        rhsA = pers.tile([P, NS], BF16)     # norm-fs bf16 duplicated both halves
        lhsC = pers.tile([P, M], BF16)      # coord lhsT rows 0:21 (see header)
        rhsC = pers.tile([P, NS], BF16)     # coord rhs rows 0:21
        thr = pers.tile([P, MT], F32)       # 100 - |ct-20|^2 per m-tile column
        ct36 = pers.tile([P, 36 * MT], BF16)  # agg lhsT: [ct_hi|1] , [ct_lo|0]
        ident = pers.tile([P, P], F32)
        biasT = pers.tile([P, 1], F32)
        scl = pers.tile([P, MT], F32)       # (2/tau) / |ft_m|

        ft_all = pers.tile([P, MT * D], F32)
        fs_all = pers.tile([P, NT * D], F32)
        ct_all = pers.tile([P, MT * 3], F32)
        cs_all = pers.tile([P, NT * 3], F32)
        s2t = pers.tile([P, MT], F32)
        s2s = pers.tile([P, NT], F32)
        ct2c = pers.tile([P, MT], F32)
        cs2c = pers.tile([P, NT], F32)
        sqf_t = pers.tile([P, MT * D], F32)
        sqf_s = pers.tile([P, NT * D], F32)
        fhl_t = pers.tile([P, MT * P], BF16)   # per tile: [hi(64) | lo(64)]
        fhh_s = pers.tile([P, NT * P], BF16)   # per tile: [hi | hi]
        # coord splits (target / source), 3 cols per tile
        ctn_all = pers.tile([P, MT * 3], F32)
        th_all = pers.tile([P, MT * 3], BF16)
        tm_all = pers.tile([P, MT * 3], BF16)
        tl_all = pers.tile([P, MT * 3], BF16)
        tr1 = pers.tile([P, MT * 3], F32)
        csc_all = pers.tile([P, NT * 3], F32)
        sh_all = pers.tile([P, NT * 3], BF16)
        sm_all = pers.tile([P, NT * 3], BF16)
        sl_all = pers.tile([P, NT * 3], BF16)
        sr1 = pers.tile([P, NT * 3], F32)
        c2h = pers.tile([P, NT], BF16)
        c2m = pers.tile([P, NT], BF16)
        c2l = pers.tile([P, NT], BF16)
        c2r = pers.tile([P, NT], F32)
        cth_all = pers.tile([P, MT * 3], BF16)
        ctl_all = pers.tile([P, MT * 3], BF16)
        rbt = pers.tile([P, MT * P], BF16)   # row-layout coord lhsT builder
        rbs = pers.tile([P, NT * P], BF16)   # (cols KC:128 zero-padded)
        resall = pers.tile([P, NT * 3], F32)

        masks.make_identity(nc, ident[:])
        nc.vector.memset(biasT[:], -scale)
        # rep-invariant zero regions (data columns are rewritten every rep,
        # zero padding is never touched) on the otherwise-idle Pool engine.
        nc.gpsimd.memset(ct36[:], 0.0)
        nc.gpsimd.memset(rbt[:], 0.0)
        nc.gpsimd.memset(rbs[:], 0.0)

        for _rep in range(repeat):
            # ---------------- load inputs ----------------
            nc.sync.dma_start(
                ft_all[:].rearrange("p (t d) -> p t d", d=D),
                ft_d.rearrange("(t p) d -> p t d", p=P))
            nc.sync.dma_start(
                fs_all[:].rearrange("p (t d) -> p t d", d=D),
                fs_d.rearrange("(t p) d -> p t d", p=P))
            nc.sync.dma_start(
                ct_all[:].rearrange("p (t c) -> p t c", c=3),
                ct_d.rearrange("(t p) c -> p t c", p=P))
            nc.sync.dma_start(
                cs_all[:].rearrange("p (t c) -> p t c", c=3),
                cs_d.rearrange("(t p) c -> p t c", p=P))

            ftv = ft_all[:].rearrange("p (t d) -> p t d", d=D)
            fsv = fs_all[:].rearrange("p (t d) -> p t d", d=D)
            ctv = ct_all[:].rearrange("p (t c) -> p t c", c=3)
            csv = cs_all[:].rearrange("p (t c) -> p t c", c=3)

            # ---------------- feature norms ----------------
            # target side: norms only (normalization folds into ACT scale)
            nc.scalar.activation(sqf_t[:], ft_all[:], AF.Square)
            nc.vector.tensor_reduce(
                s2t[:], sqf_t[:].rearrange("p (t d) -> p t d", d=D),
                axis=mybir.AxisListType.X, op=ALU.add)
            rt_ = scr.tile([P, MT], F32, tag="rt")
            nc.scalar.sqrt(rt_[:], s2t[:])
            nc.vector.reciprocal(rt_[:], rt_[:])
            nc.vector.tensor_scalar_mul(scl[:], rt_[:], scale)
            # source side: normalize for real (free dim of S^T)
            nc.scalar.activation(sqf_s[:], fs_all[:], AF.Square)
            nc.vector.tensor_reduce(
                s2s[:], sqf_s[:].rearrange("p (t d) -> p t d", d=D),
                axis=mybir.AxisListType.X, op=ALU.add)
            rn = scr.tile([P, NT], F32, tag="rn")
            nc.scalar.sqrt(rn[:], s2s[:])
            nc.vector.reciprocal(rn[:], rn[:])

            # ---------------- bf16 splits + batched transposes -----------
            # ft: exact hi/lo split of RAW ft into [128,128] blocks
            vt = fhl_t[:].rearrange("p (t k) -> p t k", k=P)
            nc.vector.tensor_copy(vt[:, :, 0:D], ftv[:])
            nc.vector.tensor_tensor(vt[:, :, D:2 * D], ftv[:], vt[:, :, 0:D],
                                    op=ALU.subtract)
            # fs: normalize-and-cast in one pass, duplicated on both halves
            vs_h = fhh_s[:].rearrange("p (t k) -> p t k", k=P)
            for t in range(NT):
                nc.vector.tensor_scalar_mul(vs_h[:, t, 0:D], fsv[:, t, :],
                                            rn[:, t:t + 1])
            nc.vector.tensor_copy(vs_h[:, :, D:2 * D], vs_h[:, :, 0:D])
            nc.sync.dma_start_transpose(
                ftT[:].rearrange("p (t c) -> p t c", c=P), fhl_t[:])
            nc.scalar.dma_start_transpose(
                rhsA[:].rearrange("p (t c) -> p t c", c=P), fhh_s[:])

            # ---------------- coordinates ----------------
            # target: ctn = -2*(ct-20) = -2*ct + 40, 3-way bf16 split
            nc.vector.tensor_scalar(ctn_all[:], ct_all[:], -2.0, 2.0 * CENTER,
                                    op0=ALU.mult, op1=ALU.add)
            nc.vector.tensor_copy(th_all[:], ctn_all[:])
            nc.vector.tensor_tensor(tr1[:], ctn_all[:], th_all[:],
                                    op=ALU.subtract)
            nc.vector.tensor_copy(tm_all[:], tr1[:])
            nc.vector.tensor_tensor(tl_all[:], tr1[:], tm_all[:],
                                    op=ALU.subtract)
            # |ct-20|^2 = |ctn|^2 / 4 ; thr = 100 - |ct-20|^2
            ctnv = ctn_all[:].rearrange("p (t c) -> p t c", c=3)
            for t in range(MT):
                sq = scr.tile([P, 3], F32, tag="sqc")
                nc.scalar.activation(sq[:], ctnv[:, t, :], AF.Square,
                                     accum_out=ct2c[:, t:t + 1])
            nc.vector.tensor_scalar(thr[:], ct2c[:], -0.25, RADIUS_SQ,
                                    op0=ALU.mult, op1=ALU.add)
            # agg lhsT: hi/lo split of UNcentered [ct | 1]
            nc.vector.tensor_copy(cth_all[:], ct_all[:])
            nc.vector.tensor_tensor(ctl_all[:], ct_all[:], cth_all[:],
                                    op=ALU.subtract)
            v36 = ct36[:].rearrange("p (t k) -> p t k", k=36)
            vh = cth_all[:].rearrange("p (t c) -> p t c", c=3)
            vl = ctl_all[:].rearrange("p (t c) -> p t c", c=3)
            nc.gpsimd.tensor_copy(v36[:, :, 0:3], vh[:])
            nc.gpsimd.memset(v36[:, :, 3:4], 1.0)
            nc.gpsimd.tensor_copy(v36[:, :, 32:35], vl[:])
            # coord lhsT row-layout builder: [h, 1, h, m, m, h, l] then T
            rt2 = rbt[:].rearrange("p (t k) -> p t k", k=P)
            vth = th_all[:].rearrange("p (t c) -> p t c", c=3)
            vtm = tm_all[:].rearrange("p (t c) -> p t c", c=3)
            vtl = tl_all[:].rearrange("p (t c) -> p t c", c=3)
            nc.gpsimd.tensor_copy(rt2[:, :, 0:3], vth[:])
            nc.gpsimd.memset(rt2[:, :, 3:6], 1.0)
            nc.gpsimd.tensor_copy(rt2[:, :, 6:9], vth[:])
            nc.gpsimd.tensor_copy(rt2[:, :, 9:12], vtm[:])
            nc.gpsimd.tensor_copy(rt2[:, :, 12:15], vtm[:])
            nc.gpsimd.tensor_copy(rt2[:, :, 15:18], vth[:])
            nc.gpsimd.tensor_copy(rt2[:, :, 18:21], vtl[:])
            nc.sync.dma_start_transpose(
                lhsC[:].rearrange("p (t c) -> p t c", c=P), rbt[:])

            # source: csc = cs - 20, 3-way split; cs2 = |csc|^2, 3-way split
            nc.vector.tensor_scalar_add(csc_all[:], cs_all[:], -CENTER)
            nc.vector.tensor_copy(sh_all[:], csc_all[:])
            nc.vector.tensor_tensor(sr1[:], csc_all[:], sh_all[:],
                                    op=ALU.subtract)
            nc.vector.tensor_copy(sm_all[:], sr1[:])
            nc.vector.tensor_tensor(sl_all[:], sr1[:], sm_all[:],
                                    op=ALU.subtract)
            cscv = csc_all[:].rearrange("p (t c) -> p t c", c=3)
            for t in range(NT):
                sq = scr.tile([P, 3], F32, tag="sqc")
                nc.scalar.activation(sq[:], cscv[:, t, :], AF.Square,
                                     accum_out=cs2c[:, t:t + 1])
            nc.vector.tensor_copy(c2h[:], cs2c[:])
            nc.vector.tensor_tensor(c2r[:], cs2c[:], c2h[:], op=ALU.subtract)
            nc.vector.tensor_copy(c2m[:], c2r[:])
            nc.vector.tensor_tensor(c2l[:], c2r[:], c2m[:], op=ALU.subtract)
            # source rows: [h, cs2h, cs2m, cs2l, m, h, m, l, h]
            rs_ = rbs[:].rearrange("p (t k) -> p t k", k=P)
            vsh = sh_all[:].rearrange("p (t c) -> p t c", c=3)
            vsm = sm_all[:].rearrange("p (t c) -> p t c", c=3)
            vsl = sl_all[:].rearrange("p (t c) -> p t c", c=3)
            rs2 = rbs[:].rearrange("p (t k) -> p k t", k=P)
            nc.gpsimd.tensor_copy(rs_[:, :, 0:3], vsh[:])
            nc.gpsimd.tensor_copy(rs2[:, 3, :], c2h[:])
            nc.gpsimd.tensor_copy(rs2[:, 4, :], c2m[:])
            nc.gpsimd.tensor_copy(rs2[:, 5, :], c2l[:])
            nc.gpsimd.tensor_copy(rs_[:, :, 6:9], vsm[:])
            nc.gpsimd.tensor_copy(rs_[:, :, 9:12], vsh[:])
            nc.gpsimd.tensor_copy(rs_[:, :, 12:15], vsm[:])
            nc.gpsimd.tensor_copy(rs_[:, :, 15:18], vsl[:])
            nc.gpsimd.tensor_copy(rs_[:, :, 18:21], vsh[:])
            nc.scalar.dma_start_transpose(
                rhsC[:].rearrange("p (t c) -> p t c", c=P), rbs[:])

            # ---------------- main loop ----------------
            for j in range(NCHUNK):
                cols = slice(j * CHUNK, (j + 1) * CHUNK)
                aggp = psG.tile([36, CHUNK], F32, tag="agg")
                for mt in range(MT):
                    msl = slice(mt * P, (mt + 1) * P)
                    sp = psA.tile([P, CHUNK], F32, tag="sp")
                    nc.tensor.matmul(sp[:], ftT[:, msl], rhsA[:, cols],
                                     start=True, stop=True)
                    rp = psB.tile([P, CHUNK], F32, tag="rp")
                    nc.tensor.matmul(rp[:], lhsC[0:KC, msl],
                                     rhsC[0:KC, cols], start=True, stop=True)
                    e = sbE.tile([P, CHUNK], F32, tag="e")
                    nc.scalar.activation(e[:], sp[:], AF.Exp, bias=biasT[:],
                                         scale=scl[:, mt:mt + 1])
                    k = sbK.tile([P, CHUNK], BF16, tag="k")
                    nc.vector.scalar_tensor_tensor(k[:], in0=rp[:],
                                                   scalar=thr[:, mt:mt + 1],
                                                   in1=e[:], op0=ALU.is_lt,
                                                   op1=ALU.mult)
                    nc.tensor.matmul(aggp[:], ct36[:, 36 * mt:36 * mt + 36],
                                     k[:], start=(mt == 0), stop=(mt == MT - 1))
                agg_hi = fin.tile([4, CHUNK], F32, tag="agghi")
                nc.vector.tensor_copy(agg_hi[:], aggp[0:4, :])
                agg_sb = fin.tile([4, CHUNK], F32, tag="aggsb")
                nc.vector.tensor_tensor(agg_sb[:], agg_hi[:],
                                        aggp[32:36, :], op=ALU.add)
                for tl in range(CHUNK // P):
                    nt = j * (CHUNK // P) + tl
                    tp = psS.tile([P, 4], F32, tag="tp")
                    nc.tensor.matmul(tp[:], agg_sb[:, tl * P:(tl + 1) * P],
                                     ident[0:4, 0:4], is_transpose=True)
                    tsb = fin.tile([P, 4], F32, tag="tsb")
                    nc.vector.tensor_copy(tsb[:], tp[:])
                    rec = fin.tile([P, 1], F32, tag="rec")
                    nc.vector.tensor_scalar_add(rec[:], tsb[:, 3:4], 1e-8)
                    nc.vector.reciprocal(rec[:], rec[:])
                    resv = resall[:].rearrange("p (t c) -> p t c", c=3)
                    nc.vector.scalar_tensor_tensor(resv[:, nt, :],
                                                   in0=tsb[:, 0:3],
                                                   scalar=rec[:],
                                                   in1=csv[:, nt, :],
                                                   op0=ALU.mult,
                                                   op1=ALU.subtract)
            nc.sync.dma_start(out_d.rearrange("(t p) c -> p t c", p=P),
                              resall[:].rearrange("p (t c) -> p t c", c=3))

    nc.compile()
    return nc


_CACHE = {}


def kernel(feat_s, feat_t, coor_s, coor_t, epsilon):
    feat_s = np.ascontiguousarray(feat_s, dtype=np.float32)
    feat_t = np.ascontiguousarray(feat_t, dtype=np.float32)
    coor_s = np.ascontiguousarray(coor_s, dtype=np.float32)
    coor_t = np.ascontiguousarray(coor_t, dtype=np.float32)
    tau = float(np.exp(np.float32(epsilon)) + np.float32(TAU_OFFSET))

    key = round(tau, 12)
    if key not in _CACHE:
        _CACHE[key] = build_kernel(tau)
    nc = _CACHE[key]

    in_maps = []
    for c in range(N_CORES):
        b = c // CORES_PER_BATCH
        r = c % CORES_PER_BATCH
        sl = slice(r * NS, (r + 1) * NS)
        in_maps.append({
            "fs": np.ascontiguousarray(feat_s[b, sl]),
            "ft": feat_t[b],
            "cs": np.ascontiguousarray(coor_s[b, sl]),
            "ct": coor_t[b],
        })
    res = run_bass_kernel_spmd(nc, in_maps, core_ids=list(range(N_CORES)))
    out = np.empty((B, N, 3), dtype=np.float32)
    for c in range(N_CORES):
        b = c // CORES_PER_BATCH
        r = c % CORES_PER_BATCH
        out[b, r * NS:(r + 1) * NS] = res.results[c]["out"]
    return out
